# revision 16
# baseline (speedup 1.0000x reference)
"""Trainium2 Bass kernel for a 4-layer LIF spiking net (snntorch-style Leaky).

Strategy (data-parallel over batch, 8 cores, 32 batch rows each):
  - Phase 0: c1[t] = x_t @ W1.T + b1 precomputed for all T (scan-independent),
    spilled to a DRAM scratch in a "folded" [128, 512] layout
    (partition = 32*j + b where j = h//512, col = h%512).
  - Scan phase (T sequential steps): layers 2..4 matmuls use the
    activation-stationary orientation: stationary = spikes^T [128, 32] tile,
    moving = W^T [128, 512] chunks, with 4-way PE column tiling
    (tile_position=(0, 32j)) so the four 512-wide N-chunks accumulate into
    the four 32-partition strips of one PSUM bank -> folded layout, full
    PE-array utilization at local batch 32.
  - LIF update runs on the folded [128, 512] tiles (all 128 partitions).
  - Spike transposes for the next layer's stationary: 4 PE transposes of
    [128, 128] blocks per layer per step + ScalarE copies PSUM->SBUF.
  - Only membrane potentials m1..m4 are written to DRAM. Spikes and currents
    are reconstructed on host: s_t = (m_t > thr), c_t = m_t - b*m_{t-1} +
    thr*s_{t-1} (exact / 1-ulp identities of the reference recurrence).
  - W2^T stays fp32 resident in SBUF; W3^T is bf16 (layer-3 error budget is
    large: m3 max ~0.76 vs thr 1.0), partially resident + streamed per step.
"""

import math
import numpy as np
import ml_dtypes

B, T, D, H, OUT = 256, 25, 768, 2048, 10
NCORES = 8
BL = B // NCORES            # 32 local batch rows
OUTP = 16                   # padded layer-4 width
KT2 = H // 128              # 16 K-tiles for H-contractions
KT1 = D // 128              # 6 K-tiles for the input contraction
NCH = H // 512              # 4 N-chunks / folds
N_W2_RES = 14               # resident fp32 W2^T K-rows; rest streamed per step


def build_nc(beta: float, thr: float, n_steps: int = T):
    import concourse.mybir as mybir
    from concourse import bacc
    from concourse.tile import TileContext
    from concourse.masks import make_identity

    f32 = mybir.dt.float32
    bf16 = mybir.dt.bfloat16
    A = mybir.AluOpType

    nc = bacc.Bacc()

    # ---- DRAM I/O ----
    xT_d = nc.dram_tensor("xT", [n_steps, D, BL], f32, kind="ExternalInput")
    w1t_d = nc.dram_tensor("w1t", [D, H], f32, kind="ExternalInput")
    w2t_d = nc.dram_tensor("w2t", [H, H], f32, kind="ExternalInput")
    w3t_d = nc.dram_tensor("w3t", [H, H], bf16, kind="ExternalInput")
    w4t_d = nc.dram_tensor("w4t", [H, OUTP], bf16, kind="ExternalInput")
    b1_d = nc.dram_tensor("b1", [1, H], f32, kind="ExternalInput")
    b2f_d = nc.dram_tensor("b2f", [128, 512], f32, kind="ExternalInput")
    b3f_d = nc.dram_tensor("b3f", [128, 512], f32, kind="ExternalInput")
    b4f_d = nc.dram_tensor("b4f", [BL, OUTP], f32, kind="ExternalInput")

    m1_d = nc.dram_tensor("m1", [n_steps, BL, H], f32, kind="ExternalOutput")
    m2_d = nc.dram_tensor("m2", [n_steps, BL, H], f32, kind="ExternalOutput")
    m3_d = nc.dram_tensor("m3", [n_steps, BL, H], f32, kind="ExternalOutput")
    m4_d = nc.dram_tensor("m4", [n_steps, BL, OUTP], f32, kind="ExternalOutput")

    c1s_d = nc.dram_tensor("c1s", [n_steps, 128, 512], f32)  # internal scratch

    with TileContext(nc) as tc:
        # ---------- resident fp32 W2^T pool (opened first: lowest addrs) ----------
        with tc.tile_pool(name="w2pool", bufs=1) as w2pool:
            w2r = []
            for k in range(N_W2_RES):
                w2r.append(w2pool.tile([128, H], f32, name=f"w2r{k}"))

            # ---------- phase 0: precompute c1[t] for all t ----------
            with (
                tc.tile_pool(name="p0", bufs=1) as p0,
                tc.tile_pool(name="p0x", bufs=3) as p0x,
                tc.tile_pool(name="p0st", bufs=3) as p0st,
                tc.tile_pool(name="p0ps", bufs=4, space="PSUM") as p0ps,
            ):
                w1r = []
                for g in range(KT1):
                    w1ti = p0.tile([128, H], f32, name=f"w1r{g}")
                    nc.sync.dma_start(w1ti, w1t_d[128 * g:128 * (g + 1), :])
                    w1r.append(w1ti)
                b1_sb = p0.tile([1, H], f32)
                nc.sync.dma_start(b1_sb, b1_d[:, :])
                ones_sb = p0.tile([1, BL], f32)
                nc.vector.memset(ones_sb, 1.0)

                for t in range(n_steps):
                    xt = p0x.tile([128, KT1 * BL], f32, tag="xt")
                    nc.sync.dma_start(
                        xt.rearrange("p (k b) -> p k b", k=KT1),
                        xT_d[t].rearrange("(k p) b -> p k b", p=128))
                    ps = p0ps.tile([128, 512], f32, tag="c1ps")
                    for j in range(NCH):
                        nc.tensor.matmul(
                            ps[32 * j:32 * (j + 1), :],
                            ones_sb[0:1, :],
                            b1_sb[0:1, 512 * j:512 * (j + 1)],
                            tile_position=(0, 32 * j),
                            start=True, stop=False,
                            skip_group_check=True,
                        )
                    for g in range(KT1):
                        for j in range(NCH):
                            nc.tensor.matmul(
                                ps[32 * j:32 * (j + 1), :],
                                xt[:, (g * BL):(g + 1) * BL],
                                w1r[g][:, 512 * j:512 * (j + 1)],
                                tile_position=(0, 32 * j),
                                start=False, stop=(g == KT1 - 1),
                                skip_group_check=True,
                            )
                    cst = p0st.tile([128, 512], f32, tag="c1st")
                    nc.scalar.copy(cst, ps)
                    nc.sync.dma_start(c1s_d[t], cst)
                    # spread the big resident W2 loads between phase-0 steps
                    if t < N_W2_RES:
                        nc.sync.dma_start(
                            w2r[t], w2t_d[128 * t:128 * (t + 1), :])
                for k in range(n_steps, N_W2_RES):
                    nc.sync.dma_start(w2r[k], w2t_d[128 * k:128 * (k + 1), :])

            # ---------- scan-phase pools ----------
            with (
                tc.tile_pool(name="wres", bufs=1) as wres,
                tc.tile_pool(name="w2st", bufs=2) as w2st,
                tc.tile_pool(name="state", bufs=1) as stp,
                tc.tile_pool(name="c1pool", bufs=2) as c1pool,
                tc.tile_pool(name="psA", bufs=2, space="PSUM") as psA,
                tc.tile_pool(name="psB", bufs=2, space="PSUM") as psB,
            ):
                w3r = []
                for k in range(KT2):
                    w3ti = wres.tile([128, H], bf16, name=f"w3r{k}")
                    nc.sync.dma_start(w3ti, w3t_d[128 * k:128 * (k + 1), :])
                    w3r.append(w3ti)
                w4_sb = wres.tile([128, KT2 * OUTP], bf16)
                for k in range(KT2):
                    nc.sync.dma_start(
                        w4_sb[:, OUTP * k:OUTP * (k + 1)],
                        w4t_d[128 * k:128 * (k + 1), :])
                b2f = wres.tile([128, 512], f32)
                nc.sync.dma_start(b2f, b2f_d[:, :])
                b3f = wres.tile([128, 512], f32)
                nc.sync.dma_start(b3f, b3f_d[:, :])
                b4f = wres.tile([BL, OUTP], f32)
                nc.sync.dma_start(b4f, b4f_d[:, :])
                ident = wres.tile([128, 128], bf16)
                make_identity(nc, ident)

                # persistent states (folded layout [128, 512])
                M1 = stp.tile([128, 512], f32)
                M2 = stp.tile([128, 512], f32)
                M3 = stp.tile([128, 512], f32)
                S1 = stp.tile([128, 512], bf16)
                S2 = stp.tile([128, 512], bf16)
                S3 = stp.tile([128, 512], bf16)
                S1T = stp.tile([128, 512], f32)
                S2T = stp.tile([128, 512], bf16)
                S3T = stp.tile([128, 512], bf16)
                M4 = stp.tile([BL, (n_steps + 1) * OUTP], f32)
                S4 = stp.tile([BL, OUTP], f32)
                for st_tile in (M1, M2, M3, S1, S2, S3, M4, S4):
                    nc.vector.memset(st_tile, 0.0)

                def lif_big(Mt, St, cur_ap, bias_ap, out_md, t):
                    # m = ((b*m - s_prev*thr) + cur) + bias ; dma out ; s = m > thr
                    if thr == 1.0:
                        nc.vector.scalar_tensor_tensor(
                            out=Mt, in0=Mt, scalar=beta, in1=St,
                            op0=A.mult, op1=A.subtract)
                    else:
                        nc.vector.tensor_scalar_mul(St, St, thr)
                        nc.vector.scalar_tensor_tensor(
                            out=Mt, in0=Mt, scalar=beta, in1=St,
                            op0=A.mult, op1=A.subtract)
                    nc.vector.tensor_tensor(
                        out=Mt, in0=Mt, in1=cur_ap, op=A.add)
                    if bias_ap is not None:
                        nc.vector.tensor_tensor(
                            out=Mt, in0=Mt, in1=bias_ap, op=A.add)
                    for j in range(NCH):
                        nc.sync.dma_start(
                            out_md[t, :, 512 * j:512 * (j + 1)],
                            Mt[32 * j:32 * (j + 1), :])
                    nc.vector.tensor_scalar(
                        out=St, in0=Mt, scalar1=thr, scalar2=None, op0=A.is_gt)

                def transposes(St, STt):
                    for q in range(4):
                        pt = psB.tile([128, 128], bf16, tag="ptr")
                        nc.tensor.transpose(
                            pt, St[:, 128 * q:128 * (q + 1)], ident)
                        nc.scalar.copy(STt[:, 128 * q:128 * (q + 1)], pt)

                def stat_ap(STt, k):
                    off = 128 * (k % 4) + 32 * (k // 4)
                    return STt[:, off:off + BL]

                for t in range(n_steps):
                    c1t = c1pool.tile([128, 512], f32, tag="c1t")
                    nc.sync.dma_start(c1t, c1s_d[t])

                    # ---- layer 1 (bias already inside c1) ----
                    lif_big(M1, S1, c1t, None, m1_d, t)
                    transposes(S1, S1T)

                    # ---- layer 2 ----
                    # mm order: resident (k, j) pairs with the streamed rows'
                    # chunks interleaved every few mms so the 2-slot w2s tag
                    # rotation never stalls the PE.
                    order = []
                    for k in range(N_W2_RES):
                        for j in range(NCH):
                            order.append((k, j))
                    pos = len(order) - 1
                    si = 0
                    for k in range(N_W2_RES, KT2):
                        for j in range(NCH):
                            order.insert(pos - 6 * si, (k, j))
                            si += 1
                    first_j = {}
                    last_j = {}
                    for k, j in order:
                        if j not in first_j:
                            first_j[j] = (k, j)
                        last_j[j] = (k, j)
                    ps2 = psA.tile([128, 512], f32, tag="c2")
                    for k, j in order:
                        if k < N_W2_RES:
                            rhs = w2r[k][:, 512 * j:512 * (j + 1)]
                        else:
                            w2sk = w2st.tile([128, 512], f32, tag="w2s")
                            nc.sync.dma_start(
                                w2sk,
                                w2t_d[128 * k:128 * (k + 1),
                                      512 * j:512 * (j + 1)])
                            rhs = w2sk[:, :]
                        nc.tensor.matmul(
                            ps2[32 * j:32 * (j + 1), :],
                            stat_ap(S1T, k),
                            rhs,
                            tile_position=(0, 32 * j),
                            start=(first_j[j] == (k, j)),
                            stop=(last_j[j] == (k, j)),
                            skip_group_check=True,
                        )
                    lif_big(M2, S2, ps2, b2f, m2_d, t)
                    transposes(S2, S2T)

                    # ---- layer 3 ----
                    ps3 = psA.tile([128, 512], f32, tag="c3")
                    for k in range(KT2):
                        w3k = w3r[k]
                        for j in range(NCH):
                            nc.tensor.matmul(
                                ps3[32 * j:32 * (j + 1), :],
                                stat_ap(S2T, k),
                                w3k[:, 512 * j:512 * (j + 1)],
                                tile_position=(0, 32 * j),
                                start=(k == 0), stop=(k == KT2 - 1),
                                skip_group_check=True,
                            )
                    lif_big(M3, S3, ps3, b3f, m3_d, t)
                    transposes(S3, S3T)

                    # ---- layer 4 ----
                    ps4 = psB.tile([BL, OUTP], f32, tag="c4")
                    for k in range(KT2):
                        nc.tensor.matmul(
                            ps4,
                            stat_ap(S3T, k),
                            w4_sb[:, OUTP * k:OUTP * (k + 1)],
                            start=(k == 0), stop=(k == KT2 - 1),
                            skip_group_check=True,
                        )
                    m4p = M4[:, OUTP * t:OUTP * (t + 1)]
                    m4n = M4[:, OUTP * (t + 1):OUTP * (t + 2)]
                    nc.vector.scalar_tensor_tensor(
                        out=m4n, in0=m4p, scalar=beta, in1=S4,
                        op0=A.mult, op1=A.subtract)
                    nc.vector.tensor_tensor(out=m4n, in0=m4n, in1=ps4, op=A.add)
                    nc.vector.tensor_tensor(out=m4n, in0=m4n, in1=b4f, op=A.add)
                    nc.vector.tensor_scalar(
                        out=S4, in0=m4n, scalar1=thr, scalar2=None, op0=A.is_gt)

                nc.sync.dma_start(
                    m4_d[:, :, :].rearrange("t b o -> b t o"),
                    M4[:, OUTP:OUTP * (n_steps + 1)].rearrange(
                        "b (t o) -> b t o", o=OUTP))

    nc.compile()
    return nc


def _prep_inputs(inputs, n_steps=T):
    """Host-side prep: slice per core, transpose/cast, build in_maps."""
    x = np.asarray(inputs["x"], dtype=np.float32)
    W1 = np.asarray(inputs["W1"], dtype=np.float32)
    W2 = np.asarray(inputs["W2"], dtype=np.float32)
    W3 = np.asarray(inputs["W3"], dtype=np.float32)
    W4 = np.asarray(inputs["W4"], dtype=np.float32)
    b1 = np.asarray(inputs["b1"], dtype=np.float32)
    b2 = np.asarray(inputs["b2"], dtype=np.float32)
    b3 = np.asarray(inputs["b3"], dtype=np.float32)
    b4 = np.asarray(inputs["b4"], dtype=np.float32)

    w1t = np.ascontiguousarray(W1.T)                       # [768, 2048] f32
    w2t = np.ascontiguousarray(W2.T)                       # [2048, 2048] f32
    w3t = np.ascontiguousarray(W3.T).astype(ml_dtypes.bfloat16)
    w4t = np.zeros((H, OUTP), dtype=ml_dtypes.bfloat16)
    w4t[:, :OUT] = W4.T.astype(ml_dtypes.bfloat16)
    b1r = b1.reshape(1, H)
    # folded biases: value at [32j+b, c] = bias[512j + c]
    b2f = np.broadcast_to(b2.reshape(NCH, 1, 512), (NCH, BL, 512)).reshape(128, 512)
    b3f = np.broadcast_to(b3.reshape(NCH, 1, 512), (NCH, BL, 512)).reshape(128, 512)
    b4f = np.zeros((BL, OUTP), dtype=np.float32)
    b4f[:, :OUT] = b4.reshape(1, OUT)

    in_maps = []
    for c in range(NCORES):
        xs = x[c * BL:(c + 1) * BL, :n_steps, :]            # [32, T, 768]
        xTc = np.ascontiguousarray(np.transpose(xs, (1, 2, 0)))  # [T, 768, 32]
        in_maps.append(dict(
            xT=xTc, w1t=w1t, w2t=w2t, w3t=w3t, w4t=w4t,
            b1=b1r, b2f=np.ascontiguousarray(b2f),
            b3f=np.ascontiguousarray(b3f), b4f=b4f,
        ))
    return in_maps


def _reconstruct(m_all, beta, thr, feat):
    """Given m [T, B, feat], rebuild s = (m > thr) and
    c = m_t - beta*m_{t-1} + thr*s_{t-1} (m_-1 = 0, s_-1 = 0)."""
    s = (m_all > thr).astype(np.float32)
    m_prev = np.concatenate(
        [np.zeros((1,) + m_all.shape[1:], np.float32), m_all[:-1]], axis=0)
    s_prev = np.concatenate(
        [np.zeros((1,) + s.shape[1:], np.float32), s[:-1]], axis=0)
    c = m_all - np.float32(beta) * m_prev + np.float32(thr) * s_prev
    return s, c


def kernel(**inputs):
    from concourse.bass_utils import run_bass_kernel_spmd

    beta_in = float(np.asarray(inputs["beta1"]))
    thr = float(np.asarray(inputs["thr1"]))
    # reference clips beta into [0, 1]
    beta = min(max(beta_in, 0.0), 1.0)
    for k in ("beta2", "beta3", "beta4"):
        assert abs(float(np.asarray(inputs[k])) - beta_in) < 1e-12
    for k in ("thr2", "thr3", "thr4"):
        assert abs(float(np.asarray(inputs[k])) - thr) < 1e-12

    nc = build_nc(beta, thr, T)
    in_maps = _prep_inputs(inputs, T)
    res = run_bass_kernel_spmd(nc, in_maps, core_ids=list(range(NCORES)))

    m1 = np.concatenate([r["m1"] for r in res.results], axis=1)  # [T, 256, 2048]
    m2 = np.concatenate([r["m2"] for r in res.results], axis=1)
    m3 = np.concatenate([r["m3"] for r in res.results], axis=1)
    m4 = np.concatenate([r["m4"] for r in res.results], axis=1)[:, :, :OUT]

    s1, c1 = _reconstruct(m1, beta, thr, H)
    s2, c2 = _reconstruct(m2, beta, thr, H)
    s3, c3 = _reconstruct(m3, beta, thr, H)
    s4, c4 = _reconstruct(m4, beta, thr, OUT)
    return (s1, s2, s3, s4, m1, m2, m3, m4, c1, c2, c3, c4)
